# revision 2
# baseline (speedup 1.0000x reference)
"""Differential multi-head attention on 8 Trainium2 NeuronCores.

Sharding: core p owns head pair (p, p+8) for both batches (tensor parallel
over the 8 differential head pairs). lambda scalars are folded into the
output-projection weights on the host. Host sums the 8 partial outputs.

Layout per core (hd = 64, pair cols = 128, T = B*N = 4096 tokens):
  xT      [1024, 4096]   x transposed (features on partitions), fp32r
  QT, KT  [128, 4096]    projected q/k transposed; rows 0:64 = head p,
                         rows 64:128 = head p+8
  V       [4096, 130]    token-partition layout, cols [h1(64) | 1 | h2(64) | 1]
  S.T     [k, q] chunks  via matmul(lhsT=KT slice, rhs=QT slice), K=64
  P.T     exp(S.T/8)     ACT, written as fp32r
  OT_aug  [65, 512]      psum accum over 16 k-chunks: rows 0:64 = (P@V).T,
                         row 64 = softmax denominators
  out.T   [1024, 4096]   = Wcomb.T @ OcombT, partial; summed on host
"""
import numpy as np

import concourse.bacc as bacc
import concourse.bass as bass
import concourse.tile as tile
import concourse.mybir as mybir
from concourse.bass_utils import run_bass_kernel_spmd

F32 = mybir.dt.float32
F32R = mybir.dt.float32r

EMBED = 1024
NHEADS = 16
H2 = 8
HD = 64
B = 2
N = 2048
T = B * N  # 4096
NCORES = 8
LAMBDA_INIT = 0.8
SCALE = HD ** -0.5

TRACE = False
LAST_RESULT = [None]

_compiled = [None]


def ts(i, size):
    return slice(i * size, (i + 1) * size)


def _build():
    nc = bacc.Bacc("TRN2", target_bir_lowering=False, debug=False, num_devices=NCORES)

    xT_d = nc.dram_tensor("xT", [EMBED, T], F32R, kind="ExternalInput").ap()
    wq_d = nc.dram_tensor("wq", [8, 128, 128], F32R, kind="ExternalInput").ap()
    wk_d = nc.dram_tensor("wk", [8, 128, 128], F32R, kind="ExternalInput").ap()
    wv_d = nc.dram_tensor("wv", [8, 128, 128], F32R, kind="ExternalInput").ap()
    wc_d = nc.dram_tensor("wcomb", [128, 1024], F32R, kind="ExternalInput").ap()
    bq_d = nc.dram_tensor("bq", [128, 1], F32, kind="ExternalInput").ap()
    bk_d = nc.dram_tensor("bk", [128, 1], F32, kind="ExternalInput").ap()
    bva_d = nc.dram_tensor("bvaug", [1, 130], F32, kind="ExternalInput").ap()
    outT_d = nc.dram_tensor("outT", [EMBED, T], F32, kind="ExternalOutput").ap()
    d_dram = nc.dram_tensor("d_scratch", [16, 512], F32).ap()
    rd_dram = nc.dram_tensor("rd_scratch", [16, 512], F32).ap()

    with tile.TileContext(nc) as tc:
        with (
            tc.tile_pool(name="consts", bufs=1) as consts,
            tc.tile_pool(name="xp", bufs=2) as xp,
            tc.tile_pool(name="qkv", bufs=1) as qkv,
            tc.tile_pool(name="ptp", bufs=2) as ptp,
            tc.tile_pool(name="stage", bufs=3) as stage,
            tc.tile_pool(name="bcp", bufs=2) as bcp,
            tc.tile_pool(name="outp", bufs=4) as outp,
            tc.tile_pool(name="ps_st", bufs=2, space="PSUM") as ps_st,
            tc.tile_pool(name="ps_ot", bufs=1, space="PSUM") as ps_ot,
        ):
            # ---- load constants ----
            wq_t = consts.tile([128, 8, 128], F32R, name="wq_t")
            wk_t = consts.tile([128, 8, 128], F32R, name="wk_t")
            wv_t = consts.tile([128, 8, 128], F32R, name="wv_t")
            wc_t = consts.tile([128, 1024], F32R, name="wc_t")
            bq_t = consts.tile([128, 1], F32, name="bq_t")
            bk_t = consts.tile([128, 1], F32, name="bk_t")
            bva_t = consts.tile([128, 130], F32, name="bva_t")
            nc.sync.dma_start(out=wq_t, in_=wq_d.rearrange("c p m -> p c m"))
            nc.sync.dma_start(out=wk_t, in_=wk_d.rearrange("c p m -> p c m"))
            nc.sync.dma_start(out=wv_t, in_=wv_d.rearrange("c p m -> p c m"))
            nc.sync.dma_start(out=wc_t, in_=wc_d)
            nc.sync.dma_start(out=bq_t, in_=bq_d)
            nc.sync.dma_start(out=bk_t, in_=bk_d)
            nc.sync.dma_start(
                out=bva_t,
                in_=bass.AP(tensor=bva_d.tensor, offset=0,
                            ap=[[0, 128]] + list(bva_d.ap[-1:])),
            )

            qt_t = qkv.tile([128, T], F32R, name="qt_t")
            kt_t = qkv.tile([128, T], F32R, name="kt_t")
            v_t = qkv.tile([128, 32, 130], F32R, name="v_t")
            ot_t = qkv.tile([128, B, N], F32, name="ot_t")
            oc_t = qkv.tile([128, B, N], F32R, name="oc_t")
            d_all = consts.tile([16, 512], F32, name="d_all")
            rd_all = consts.tile([16, 512], F32, name="rd_all")
            rscr = consts.tile([16, 512], F32, name="rscr")

            xT_r = xT_d.rearrange("(c p) n -> p c n", p=128)

            # ---- phase P: projections ----
            for t in range(8):  # token chunks of 512
                xt = xp.tile([128, 8, 512], F32R, name="xt")
                nc.sync.dma_start(out=xt, in_=xT_r[:, :, ts(t, 512)])
                for wt, dst, bias in ((wq_t, qt_t, bq_t), (wk_t, kt_t, bk_t)):
                    psq = ps_st.tile([128, 1536], F32, name="ps_st")
                    for f in range(8):
                        nc.tensor.matmul(
                            psq[:, 0:512], wt[:, f, :], xt[:, f, :],
                            start=(f == 0), stop=(f == 7),
                        )
                    # copy + bias (per-partition scalar) via ACT, rounds to f32r
                    nc.scalar.activation(
                        dst[:, ts(t, 512)], psq[:, 0:512],
                        mybir.ActivationFunctionType.Identity, bias=bias,
                    )
                for sub in range(4):  # token sub-chunks of 128
                    c = t * 4 + sub
                    psv = ps_st.tile([128, 1536], F32, name="ps_st")
                    for f in range(8):
                        nc.tensor.matmul(
                            psv[:, 0:128], xt[:, f, ts(sub, 128)], wv_t[:, f, :],
                            start=(f == 0), stop=(f == 7),
                        )
                    nc.vector.tensor_add(v_t[:, c, 0:64], psv[:, 0:64], bva_t[:, 0:64])
                    nc.vector.tensor_add(v_t[:, c, 65:129], psv[:, 64:128], bva_t[:, 65:129])
            # ones columns for the denominator rows
            nc.vector.tensor_copy(
                v_t[:, :, 64:65], bva_t[:, None, 64:65].broadcast_to([128, 32, 1])
            )
            nc.vector.tensor_copy(
                v_t[:, :, 129:130], bva_t[:, None, 129:130].broadcast_to([128, 32, 1])
            )

            # ---- phase A: attention ----
            for b in range(2):
                for qc in range(4):  # q chunks of 512
                    otps = [
                        ps_ot.tile([65, 512], F32, name=f"ps_ot{h}") for h in (0, 1)
                    ]
                    qoff = b * N + qc * 512
                    slots = [(kc, h) for kc in range(16) for h in (0, 1)]
                    for g0 in range(0, 32, 3):
                        grp = slots[g0:g0 + 3]
                        st = ps_st.tile([128, 1536], F32, name="ps_st")
                        pt = ptp.tile([128, 1536], F32R, name="pt")
                        for i, (kc, h) in enumerate(grp):
                            lo = h * 64
                            koff = b * N + kc * 128
                            nc.tensor.matmul(
                                st[:, ts(i, 512)],
                                kt_t[lo:lo + 64, koff:koff + 128],
                                qt_t[lo:lo + 64, qoff:qoff + 512],
                                start=True, stop=True,
                            )
                        w = len(grp) * 512
                        nc.scalar.activation(
                            pt[:, 0:w], st[:, 0:w],
                            mybir.ActivationFunctionType.Exp, scale=SCALE,
                        )
                        for i, (kc, h) in enumerate(grp):
                            nc.tensor.matmul(
                                otps[h],
                                v_t[:, b * 16 + kc, h * 65:(h + 1) * 65],
                                pt[:, ts(i, 512)],
                                start=(kc == 0), stop=(kc == 15),
                            )
                    for h in (0, 1):
                        idx = 8 * h + b * 4 + qc
                        stg = stage.tile([65, 512], F32, name="stg")
                        nc.vector.tensor_copy(stg, otps[h])
                        nc.sync.dma_start(
                            out=ot_t[h * 64:(h + 1) * 64, b, ts(qc, 512)],
                            in_=stg[0:64, :],
                        )
                        nc.sync.dma_start(out=d_dram[idx:idx + 1, :], in_=stg[64:65, :])

            # ---- phase B: normalize ----
            nc.sync.dma_start(out=d_all, in_=d_dram)
            nc.vector.reciprocal_approx_accurate(rd_all, d_all, rscr)
            nc.sync.dma_start(out=rd_dram, in_=rd_all)
            for b in range(2):
                for qc in range(4):
                    bc = bcp.tile([128, 512], F32, name="bc")
                    for h in (0, 1):
                        idx = 8 * h + b * 4 + qc
                        nc.sync.dma_start(
                            out=bc[h * 64:(h + 1) * 64, :],
                            in_=bass.AP(tensor=rd_dram.tensor, offset=idx * 512,
                                        ap=[[0, 64], [1, 512]]),
                        )
                    nc.vector.tensor_mul(
                        oc_t[:, b, ts(qc, 512)], ot_t[:, b, ts(qc, 512)], bc
                    )

            # ---- phase C: output projection ----
            for b in range(2):
                for n in range(4):
                    for m in range(8):  # out-col chunks of 128
                        pso = ps_st.tile([128, 1536], F32, name="ps_st")
                        nc.tensor.matmul(
                            pso[:, 0:512], wc_t[:, ts(m, 128)], oc_t[:, b, ts(n, 512)],
                            start=True, stop=True,
                        )
                        so = outp.tile([128, 512], F32, name="so")
                        if m % 2 == 0:
                            nc.vector.tensor_copy(so, pso[:, 0:512])
                        else:
                            nc.scalar.copy(so, pso[:, 0:512])
                        nc.sync.dma_start(
                            out=outT_d[ts(m, 128), b * N + n * 512: b * N + (n + 1) * 512],
                            in_=so,
                        )

    nc.compile()
    return nc


def kernel(x, Wq, bq, Wk, bk, Wv, bv, Wp, bp,
           lambda_q1, lambda_k1, lambda_q2, lambda_k2):
    x = np.asarray(x, dtype=np.float32)
    Wq, Wk, Wv, Wp = [np.asarray(w, dtype=np.float32) for w in (Wq, Wk, Wv, Wp)]
    bq, bk, bv, bp = [np.asarray(v, dtype=np.float32) for v in (bq, bk, bv, bp)]

    l1 = np.exp(np.minimum(
        (np.asarray(lambda_q1, np.float32) * np.asarray(lambda_k1, np.float32))
        .sum((-1, -2)), 5.0))
    l2 = np.exp(np.minimum(
        (np.asarray(lambda_q2, np.float32) * np.asarray(lambda_k2, np.float32))
        .sum((-1, -2)), 5.0))
    lv = np.float32((l1 - l2 + np.float32(LAMBDA_INIT)).mean())

    xT = np.ascontiguousarray(x.reshape(T, EMBED).T)

    if _compiled[0] is None:
        _compiled[0] = _build()
    nc = _compiled[0]

    in_maps = []
    for p in range(NCORES):
        r1 = slice(p * HD, (p + 1) * HD)          # head p rows/cols
        r2 = slice((8 + p) * HD, (9 + p) * HD)    # head p+8 rows/cols
        wq_p = np.concatenate([Wq[r1], Wq[r2]], 0).T      # [1024, 128]
        wk_p = np.concatenate([Wk[r1], Wk[r2]], 0).T
        wv_p = np.concatenate([Wv[r1], Wv[r2]], 0).T
        wpt1 = Wp[:, r1].T                                 # [64, 1024]
        wpt2 = Wp[:, r2].T
        wcomb = np.concatenate([wpt1, wpt2 - lv * wpt1], 0)  # [128, 1024]
        bva = np.concatenate(
            [bv[r1], [1.0], bv[r2], [1.0]]).astype(np.float32)[None, :]
        in_maps.append({
            "xT": xT,
            "wq": np.ascontiguousarray(wq_p.reshape(8, 128, 128)),
            "wk": np.ascontiguousarray(wk_p.reshape(8, 128, 128)),
            "wv": np.ascontiguousarray(wv_p.reshape(8, 128, 128)),
            "wcomb": np.ascontiguousarray(wcomb),
            "bq": np.concatenate([bq[r1], bq[r2]])[:, None].copy(),
            "bk": np.concatenate([bk[r1], bk[r2]])[:, None].copy(),
            "bvaug": np.ascontiguousarray(bva),
        })

    res = run_bass_kernel_spmd(
        nc, in_maps, core_ids=list(range(NCORES)), trace=TRACE,
    )
    LAST_RESULT[0] = res

    outT = res.results[0]["outT"].astype(np.float64)
    for c in range(1, NCORES):
        outT += res.results[c]["outT"]
    out = outT.T.reshape(B, N, EMBED).astype(np.float32) + bp[None, None, :]
    return out


# revision 5
# speedup vs baseline: 1.1131x; 1.1131x over previous
"""Differential multi-head attention on 8 Trainium2 NeuronCores.

Sharding: core p owns head pair (p, p+8) for both batches (tensor parallel
over the 8 differential head pairs). lambda scalars are folded into the
output-projection weights on the host. Host sums the 8 partial outputs.

Layout per core (hd = 64, pair cols = 128, T = B*N = 4096 tokens):
  xT      [1024, 4096]   x transposed (features on partitions), fp16
  QT, KT  [128, 4096]    projected q/k transposed; rows 0:64 = head p,
                         rows 64:128 = head p+8
  V       [4096, 130]    token-partition layout, cols [h1(64) | 1 | h2(64) | 1]
  S.T     [k, q] chunks  via matmul(lhsT=KT slice, rhs=QT slice), K=64
  P.T     exp(S.T/8)     ACT, written as fp16
  OT_aug  [65, 512]      psum accum over 16 k-chunks: rows 0:64 = (P@V).T,
                         row 64 = softmax denominators
  out.T   [1024, 4096]   = Wcomb.T @ OcombT, partial (fp16); summed on host
"""
import numpy as np

import concourse.bacc as bacc
import concourse.bass as bass
import concourse.tile as tile
import concourse.mybir as mybir
from concourse.bass_utils import run_bass_kernel_spmd

F32 = mybir.dt.float32
F16 = mybir.dt.float16

EMBED = 1024
H2 = 8
HD = 64
B = 2
N = 2048
T = B * N  # 4096
NCORES = 8
LAMBDA_INIT = 0.8
SCALE = HD ** -0.5

TRACE = False
LAST_RESULT = [None]

_compiled = [None]


def ts(i, size):
    return slice(i * size, (i + 1) * size)


def _build():
    nc = bacc.Bacc("TRN2", target_bir_lowering=False, debug=False, num_devices=NCORES)

    xT_d = nc.dram_tensor("xT", [EMBED, T], F16, kind="ExternalInput").ap()
    wq_d = nc.dram_tensor("wq", [8, 128, 128], F16, kind="ExternalInput").ap()
    wk_d = nc.dram_tensor("wk", [8, 128, 128], F16, kind="ExternalInput").ap()
    wv_d = nc.dram_tensor("wv", [8, 128, 128], F16, kind="ExternalInput").ap()
    wc_d = nc.dram_tensor("wcomb", [128, 1024], F16, kind="ExternalInput").ap()
    bq_d = nc.dram_tensor("bq", [128, 1], F32, kind="ExternalInput").ap()
    bk_d = nc.dram_tensor("bk", [128, 1], F32, kind="ExternalInput").ap()
    bva_d = nc.dram_tensor("bvaug", [1, 130], F32, kind="ExternalInput").ap()
    outT_d = nc.dram_tensor("outT", [EMBED, T], F16, kind="ExternalOutput").ap()
    d_dram = nc.dram_tensor("d_scratch", [64, 512], F32).ap()
    rd_dram = nc.dram_tensor("rd_scratch", [64, 512], F32).ap()

    with tile.TileContext(nc) as tc:
        with (
            tc.tile_pool(name="consts", bufs=1) as consts,
            tc.tile_pool(name="xp", bufs=2) as xp,
            tc.tile_pool(name="qkv", bufs=1) as qkv,
            tc.tile_pool(name="ptp", bufs=2) as ptp,
            tc.tile_pool(name="stage", bufs=3) as stage,
            tc.tile_pool(name="bcp", bufs=2) as bcp,
            tc.tile_pool(name="outp", bufs=4) as outp,
            tc.tile_pool(name="ps_st", bufs=2, space="PSUM") as ps_st,
            tc.tile_pool(name="ps_ot", bufs=1, space="PSUM") as ps_ot,
        ):
            # ---- load constants ----
            wq_t = consts.tile([128, 8, 128], F16, name="wq_t")
            wk_t = consts.tile([128, 8, 128], F16, name="wk_t")
            wv_t = consts.tile([128, 8, 128], F16, name="wv_t")
            wc_t = consts.tile([128, 1024], F16, name="wc_t")
            bq_t = consts.tile([128, 1], F32, name="bq_t")
            bk_t = consts.tile([128, 1], F32, name="bk_t")
            bva_t = consts.tile([128, 130], F32, name="bva_t")
            nc.sync.dma_start(out=wq_t, in_=wq_d.rearrange("c p m -> p c m"))
            nc.sync.dma_start(out=wk_t, in_=wk_d.rearrange("c p m -> p c m"))
            nc.sync.dma_start(out=wv_t, in_=wv_d.rearrange("c p m -> p c m"))
            nc.sync.dma_start(out=wc_t, in_=wc_d)
            nc.sync.dma_start(out=bq_t, in_=bq_d)
            nc.sync.dma_start(out=bk_t, in_=bk_d)
            nc.sync.dma_start(
                out=bva_t,
                in_=bass.AP(tensor=bva_d.tensor, offset=0,
                            ap=[[0, 128]] + list(bva_d.ap[-1:])),
            )

            qt_t = qkv.tile([128, T], F16, name="qt_t")
            kt_t = qkv.tile([128, T], F16, name="kt_t")
            v_t = qkv.tile([128, 32, 130], F16, name="v_t")
            ot_t = qkv.tile([128, B, N], F32, name="ot_t")
            oc_t = qkv.tile([128, B, N], F16, name="oc_t")

            xT_r = xT_d.rearrange("(c p) n -> p c n", p=128)

            # ---- phase P: projections ----
            for t in range(8):  # token chunks of 512
                xt = xp.tile([128, 8, 512], F16, name="xt")
                nc.sync.dma_start(out=xt, in_=xT_r[:, :, ts(t, 512)])
                for wt, dst, bias in ((wq_t, qt_t, bq_t), (wk_t, kt_t, bk_t)):
                    psq = ps_st.tile([128, 1536], F32, name="ps_st")
                    for f in range(8):
                        nc.tensor.matmul(
                            psq[:, 0:512], wt[:, f, :], xt[:, f, :],
                            start=(f == 0), stop=(f == 7),
                        )
                    # copy + per-partition bias via DVE, rounds to fp16
                    nc.vector.tensor_scalar_add(
                        dst[:, ts(t, 512)], psq[:, 0:512], bias,
                    )
                for sub in range(4):  # token sub-chunks of 128
                    c = t * 4 + sub
                    psv = ps_st.tile([128, 1536], F32, name="ps_st")
                    for f in range(8):
                        nc.tensor.matmul(
                            psv[:, 0:128], xt[:, f, ts(sub, 128)], wv_t[:, f, :],
                            start=(f == 0), stop=(f == 7),
                        )
                    nc.vector.tensor_add(v_t[:, c, 0:64], psv[:, 0:64], bva_t[:, 0:64])
                    nc.vector.tensor_add(v_t[:, c, 65:129], psv[:, 64:128], bva_t[:, 65:129])
            # ones columns for the denominator rows
            nc.vector.tensor_copy(
                v_t[:, :, 64:65], bva_t[:, None, 64:65].broadcast_to([128, 32, 1])
            )
            nc.vector.tensor_copy(
                v_t[:, :, 129:130], bva_t[:, None, 129:130].broadcast_to([128, 32, 1])
            )

            # ---- per batch: attention (A), normalize (B), out-proj (C) ----
            for b in range(2):
                # A: attention
                for qc in range(4):  # q chunks of 512
                    otps = [
                        ps_ot.tile([65, 512], F32, name=f"ps_ot{h}") for h in (0, 1)
                    ]
                    qoff = b * N + qc * 512
                    slots = [(kc, h) for kc in range(16) for h in (0, 1)]
                    for g0 in range(0, 32, 3):
                        grp = slots[g0:g0 + 3]
                        st = ps_st.tile([128, 1536], F32, name="ps_st")
                        pt = ptp.tile([128, 1536], F16, name="pt")
                        for i, (kc, h) in enumerate(grp):
                            lo = h * 64
                            koff = b * N + kc * 128
                            nc.tensor.matmul(
                                st[:, ts(i, 512)],
                                kt_t[lo:lo + 64, koff:koff + 128],
                                qt_t[lo:lo + 64, qoff:qoff + 512],
                                start=True, stop=True,
                            )
                        w = len(grp) * 512
                        nc.scalar.activation(
                            pt[:, 0:w], st[:, 0:w],
                            mybir.ActivationFunctionType.Exp, scale=SCALE,
                        )
                        for i, (kc, h) in enumerate(grp):
                            nc.tensor.matmul(
                                otps[h],
                                v_t[:, b * 16 + kc, h * 65:(h + 1) * 65],
                                pt[:, ts(i, 512)],
                                start=(kc == 0), stop=(kc == 15),
                            )
                    for h in (0, 1):
                        idx = b * 32 + h * 4 + qc
                        stg = stage.tile([65, 512], F32, name="stg")
                        nc.vector.tensor_copy(stg, otps[h])
                        nc.sync.dma_start(
                            out=ot_t[h * 64:(h + 1) * 64, b, ts(qc, 512)],
                            in_=stg[0:64, :],
                        )
                        nc.sync.dma_start(out=d_dram[idx:idx + 1, :], in_=stg[64:65, :])

                # B: normalize this batch (custom DVE ops need base partition 0)
                bslc = slice(b * 32, b * 32 + 8)
                d_b = bcp.tile([8, 512], F32, name="d_b")
                rd_b = bcp.tile([8, 512], F32, name="rd_b")
                rs_b = bcp.tile([8, 512], F32, name="rs_b")
                nc.sync.dma_start(out=d_b, in_=d_dram[bslc, :])
                nc.vector.reciprocal_approx_accurate(rd_b, d_b, rs_b)
                nc.sync.dma_start(out=rd_dram[bslc, :], in_=rd_b)
                for qc in range(4):
                    bc = bcp.tile([128, 512], F32, name="bc")
                    for h in (0, 1):
                        idx = b * 32 + h * 4 + qc
                        nc.sync.dma_start(
                            out=bc[h * 64:(h + 1) * 64, :],
                            in_=bass.AP(tensor=rd_dram.tensor, offset=idx * 512,
                                        ap=[[0, 64], [1, 512]]),
                        )
                    nc.vector.tensor_mul(
                        oc_t[:, b, ts(qc, 512)], ot_t[:, b, ts(qc, 512)], bc
                    )

                # C: output projection for this batch
                for n in range(4):
                    for m in range(8):  # out-col chunks of 128
                        pso = ps_st.tile([128, 1536], F32, name="ps_st")
                        nc.tensor.matmul(
                            pso[:, 0:512], wc_t[:, ts(m, 128)], oc_t[:, b, ts(n, 512)],
                            start=True, stop=True,
                        )
                        so = outp.tile([128, 512], F16, name="so")
                        nc.vector.tensor_copy(so, pso[:, 0:512])
                        nc.sync.dma_start(
                            out=outT_d[ts(m, 128), b * N + n * 512: b * N + (n + 1) * 512],
                            in_=so,
                        )

    nc.compile()
    return nc


def kernel(x, Wq, bq, Wk, bk, Wv, bv, Wp, bp,
           lambda_q1, lambda_k1, lambda_q2, lambda_k2):
    x = np.asarray(x, dtype=np.float32)
    Wq, Wk, Wv, Wp = [np.asarray(w, dtype=np.float32) for w in (Wq, Wk, Wv, Wp)]
    bq, bk, bv, bp = [np.asarray(v, dtype=np.float32) for v in (bq, bk, bv, bp)]

    l1 = np.exp(np.minimum(
        (np.asarray(lambda_q1, np.float32) * np.asarray(lambda_k1, np.float32))
        .sum((-1, -2)), 5.0))
    l2 = np.exp(np.minimum(
        (np.asarray(lambda_q2, np.float32) * np.asarray(lambda_k2, np.float32))
        .sum((-1, -2)), 5.0))
    lv = np.float32((l1 - l2 + np.float32(LAMBDA_INIT)).mean())

    xT = np.ascontiguousarray(x.reshape(T, EMBED).T.astype(np.float16))

    if _compiled[0] is None:
        _compiled[0] = _build()
    nc = _compiled[0]

    in_maps = []
    for p in range(NCORES):
        r1 = slice(p * HD, (p + 1) * HD)          # head p rows/cols
        r2 = slice((8 + p) * HD, (9 + p) * HD)    # head p+8 rows/cols
        wq_p = np.concatenate([Wq[r1], Wq[r2]], 0).T      # [1024, 128]
        wk_p = np.concatenate([Wk[r1], Wk[r2]], 0).T
        wv_p = np.concatenate([Wv[r1], Wv[r2]], 0).T
        wpt1 = Wp[:, r1].T                                 # [64, 1024]
        wpt2 = Wp[:, r2].T
        wcomb = np.concatenate([wpt1, wpt2 - lv * wpt1], 0)  # [128, 1024]
        bva = np.concatenate(
            [bv[r1], [1.0], bv[r2], [1.0]]).astype(np.float32)[None, :]
        in_maps.append({
            "xT": xT,
            "wq": np.ascontiguousarray(wq_p.reshape(8, 128, 128).astype(np.float16)),
            "wk": np.ascontiguousarray(wk_p.reshape(8, 128, 128).astype(np.float16)),
            "wv": np.ascontiguousarray(wv_p.reshape(8, 128, 128).astype(np.float16)),
            "wcomb": np.ascontiguousarray(wcomb.astype(np.float16)),
            "bq": np.concatenate([bq[r1], bq[r2]])[:, None].copy(),
            "bk": np.concatenate([bk[r1], bk[r2]])[:, None].copy(),
            "bvaug": np.ascontiguousarray(bva),
        })

    res = run_bass_kernel_spmd(
        nc, in_maps, core_ids=list(range(NCORES)), trace=TRACE,
    )
    LAST_RESULT[0] = res

    outT = res.results[0]["outT"].astype(np.float64)
    for c in range(1, NCORES):
        outT += res.results[c]["outT"]
    out = outT.T.reshape(B, N, EMBED).astype(np.float32) + bp[None, None, :]
    return out
